# revision 1
# baseline (speedup 1.0000x reference)
"""Trainium2 Bass kernel for MultiHeadAttention (B=2, N=2048, DIM=1024, H=16).

Sharding: 8 cores = 2 batches x 4 head-groups (4 heads each).
Each core computes qkv projections for its head slice, attention, and a
partial output projection (over its 256 d-columns). Host sums the 4
partials per batch and adds the bias.

Device layout (all fp32, matmuls in float32r for full PE rate):
  xT  [d=1024, n=2048]  feature-major activations (pre-transposed on host)
  wT  [d=1024, e=768]   qkv weight slice, transposed on host (e = q|k|v 256 each)
  woT [d=256,  e=1024]  out-proj weight slice, transposed on host
  qkT [e=512, n]        q,k feature-major (on chip)
  vv  token-major V with a ones-column per head (softmax denominator comes
      out of the attn@V matmul as one extra output row)
  Sᵀ tiles [j, i]; exp on ACT in 2048-wide strips; normalization via a
  rank-1 PE broadcast of 1/denom and a DVE multiply.
"""

import os
import sys
from contextlib import ExitStack

import numpy as np

for _p in ("/opt/trn_rl_repo", os.path.expanduser("~/.axon_site/_ro/trn_rl_repo")):
    if os.path.isdir(_p) and _p not in sys.path:
        sys.path.append(_p)

import concourse.bass as bass  # noqa: E402
import concourse.mybir as mybir  # noqa: E402
import concourse.tile as tile  # noqa: E402

F32 = mybir.dt.float32
F32R = mybir.dt.float32r
EXP = mybir.ActivationFunctionType.Exp

B, N, DIM, HEADS = 2, 2048, 1024, 16
DH = DIM // HEADS          # 64
NHL = 4                    # heads per core
SCALE = DH ** -0.5
NCORES = 8
E3 = 3 * NHL * DH          # 768 qkv features per core
EV = NHL * DH              # 256 v features per core
VW = DH + 2                # 66: v + ones column + pad (fp32r needs even M)
NT = N // 128              # 16 token chunks
DC = DIM // 128            # 8 d chunks


def _r(ap):
    return ap.bitcast(F32R)


def build_nc(repeat=1, split_waits=True):
    nc = bass.Bass("TRN2", target_bir_lowering=False, debug=False,
                   num_devices=NCORES)
    xT_d = nc.dram_tensor("xT", [DIM, N], F32, kind="ExternalInput").ap()
    wT_d = nc.dram_tensor("wT", [DIM, E3], F32, kind="ExternalInput").ap()
    woT_d = nc.dram_tensor("woT", [EV, DIM], F32, kind="ExternalInput").ap()
    out_d = nc.dram_tensor("out", [N, DIM], F32, kind="ExternalOutput").ap()

    with tile.TileContext(nc) as tc, ExitStack() as ctx:
        if repeat > 1:
            ctx.enter_context(tc.For_i(0, repeat, 1))
        pers = ctx.enter_context(tc.tile_pool(name="pers", bufs=1))
        xT = pers.tile([128, DC * N], F32, tag="xT", name="xT_sb")
        wT = pers.tile([128, DC * E3], F32, tag="wT", name="wT_sb")
        woT = pers.tile([128, 2 * DIM], F32, tag="woT", name="woT_sb")
        qkT = pers.tile([128, 4 * N], F32, tag="qkT", name="qkT_sb")
        vv = pers.tile([128, NT * NHL * VW], F32, tag="vv", name="vv_sb")
        ones = pers.tile([128, 64], F32, tag="ones", name="ones_sb")

        strip_p = ctx.enter_context(tc.tile_pool(name="strip", bufs=3))
        oT_p = ctx.enter_context(tc.tile_pool(name="oT", bufs=2))
        ostg_p = ctx.enter_context(tc.tile_pool(name="ostg", bufs=2))
        rec_p = ctx.enter_context(tc.tile_pool(name="rec", bufs=2))

        st_ps = ctx.enter_context(tc.tile_pool(name="st_ps", bufs=2, space="PSUM"))
        mm_ps = ctx.enter_context(tc.tile_pool(name="mm_ps", bufs=2, space="PSUM"))
        acc_ps = ctx.enter_context(tc.tile_pool(name="acc_ps", bufs=2, space="PSUM"))

        # --- loads: DMA into staging (strip-pool slots), then DVE
        # round-copy into the fp32r-consumed persistent tensors (the BIR
        # verifier requires every writer of an fp32r matmul operand to be
        # a rounding producer, which a DMA is not) ---
        ld_p = ctx.enter_context(tc.tile_pool(name="ld", bufs=3))

        def load_rounded(dst_ap, src_ap, cols):
            stg = ld_p.tile([128, cols], F32, tag="ld", name="ld_stg")
            nc.sync.dma_start(out=stg[:], in_=src_ap)
            nc.vector.tensor_copy(_r(dst_ap), stg[:])

        for dc in range(DC):
            load_rounded(wT[:, dc * E3:(dc + 1) * E3],
                         wT_d[dc * 128:(dc + 1) * 128, :], E3)
            for p2 in range(2):
                load_rounded(
                    xT[:, dc * N + p2 * 1024: dc * N + (p2 + 1) * 1024],
                    xT_d[dc * 128:(dc + 1) * 128, p2 * 1024:(p2 + 1) * 1024],
                    1024)
        for d2 in range(2):
            load_rounded(woT[:, d2 * DIM:(d2 + 1) * DIM],
                         woT_d[d2 * 128:(d2 + 1) * 128, :], DIM)
        one_stg = ld_p.tile([128, NHL * VW * 4], F32, tag="ld", name="one_stg")
        nc.vector.memset(one_stg[:], 1.0)
        for q in range(4):
            nc.vector.tensor_copy(
                _r(vv[:, q * NHL * VW * 4:(q + 1) * NHL * VW * 4]), one_stg[:])
        nc.vector.tensor_copy(_r(ones[:]), one_stg[:, 0:64])

        # --- q,k projections: qkT[ec][n], ec0,1=q heads01,23; ec2,3=k ---
        def qk_mm(ps_ap, ec, nt4, dc):
            nc.tensor.matmul(
                ps_ap,
                _r(wT[:, dc * E3 + ec * 128: dc * E3 + (ec + 1) * 128]),
                _r(xT[:, dc * N + nt4 * 512: dc * N + (nt4 + 1) * 512]),
                start=(dc == 0), stop=(dc == DC - 1))

        def qk_proj(ec):
            for nt4 in range(4):
                ps = mm_ps.tile([128, 512], F32, tag="mm", name="qk_ps")
                for dc in range(DC):
                    qk_mm(ps[:], ec, nt4, dc)
                nc.vector.tensor_copy(
                    _r(qkT[:, ec * N + nt4 * 512: ec * N + (nt4 + 1) * 512]),
                    ps[:])

        def v_proj(nt):
            ps = acc_ps.tile([128, 512], F32, tag="acc", name="v_ps")
            for dc in range(DC):
                nc.tensor.matmul(
                    ps[:, 0:EV],
                    _r(xT[:, dc * N + nt * 128: dc * N + (nt + 1) * 128]),
                    _r(wT[:, dc * E3 + 512: dc * E3 + 768]),
                    start=(dc == 0), stop=(dc == DC - 1))
            for h in range(NHL):
                nc.vector.tensor_copy(
                    _r(vv[:, nt * NHL * VW + h * VW: nt * NHL * VW + h * VW + DH]),
                    ps[:, h * DH:(h + 1) * DH])

        # streaming first pass over ec2 (k heads 0,1) and ec0 (q heads 0,1):
        # 8 psum accumulators consume each (xT, wT) d-chunk as it lands, so
        # the PE starts ~6us in instead of waiting for the full 11MB load.
        st_a = st_ps.tile([128, 1024], F32, tag="st", name="qs_a")
        st_b = st_ps.tile([128, 1024], F32, tag="st", name="qs_b")
        mm_g = [mm_ps.tile([128, 512], F32, tag="mm", name="qs_m") for _ in range(2)]
        acc_g = [acc_ps.tile([128, 512], F32, tag="acc", name="qs_c") for _ in range(2)]
        for dc in range(DC):
            qk_mm(st_a[:, 0:512], 2, 0, dc)
            qk_mm(st_a[:, 512:1024], 2, 1, dc)
            qk_mm(st_b[:, 0:512], 2, 2, dc)
            qk_mm(st_b[:, 512:1024], 2, 3, dc)
            qk_mm(mm_g[0][:], 0, 0, dc)
            qk_mm(mm_g[1][:], 0, 1, dc)
            qk_mm(acc_g[0][:], 0, 2, dc)
            qk_mm(acc_g[1][:], 0, 3, dc)
        nc.vector.tensor_copy(_r(qkT[:, 2 * N + 0: 2 * N + 1024]), st_a[:])
        nc.vector.tensor_copy(_r(qkT[:, 2 * N + 1024: 2 * N + 2048]), st_b[:])
        for i, g in enumerate(mm_g):
            nc.vector.tensor_copy(_r(qkT[:, i * 512:(i + 1) * 512]), g[:])
        for i, g in enumerate(acc_g):
            nc.vector.tensor_copy(_r(qkT[:, (i + 2) * 512:(i + 3) * 512]), g[:])

        # --- attention + out-projection, per i-tile of 512 queries ---
        def oproj_one(oT_prev, it_prev, ng, eh):
            ps = mm_ps.tile([128, 512], F32, tag="mm", name="op_ps")
            for d2 in range(2):
                nc.tensor.matmul(
                    ps[:],
                    _r(oT_prev[:, d2 * 512 + ng * 128: d2 * 512 + (ng + 1) * 128]),
                    _r(woT[:, d2 * DIM + eh * 512: d2 * DIM + (eh + 1) * 512]),
                    start=(d2 == 0), stop=(d2 == 1))
            stg = ostg_p.tile([128, 512], F32, tag="ostg")
            nc.vector.tensor_copy(stg[:], ps[:])
            nc.sync.dma_start(
                out=out_d[it_prev * 512 + ng * 128: it_prev * 512 + (ng + 1) * 128,
                          eh * 512:(eh + 1) * 512],
                in_=stg[:])

        v_proj(0)
        v_proj(1)
        vq = [lambda nt=nt: v_proj(nt) for nt in range(2, NT)]

        def _g(ec, nt4):
            ps = mm_ps.tile([128, 512], F32, tag="mm", name="qk_ps")
            for dc in range(DC):
                qk_mm(ps[:], ec, nt4, dc)
            nc.vector.tensor_copy(
                _r(qkT[:, ec * N + nt4 * 512: ec * N + (nt4 + 1) * 512]), ps[:])

        # remaining q,k groups, ordered so each lands just before its first
        # consumer when popped every other sg across it0 h1+h2
        ecq = [lambda: _g(1, 0), lambda: _g(3, 0), lambda: _g(3, 1),
               lambda: _g(3, 2), lambda: _g(3, 3), lambda: _g(1, 1),
               lambda: _g(1, 2), lambda: _g(1, 3)]

        pending = []  # deferred out-proj work, interleaved into strip loop
        normq = []    # deferred per-head normalize chains

        def fill_hook(it, h, sg):
            if it == 0:
                if h == 0:
                    for _ in range(2):
                        if vq:
                            vq.pop(0)()
                elif h in (1, 2):
                    if sg % 2 == 0 and ecq:
                        ecq.pop(0)()
            elif sg % 4 == 3 and pending:
                pending.pop(0)()

        for it in range(4):
            oT_t = oT_p.tile([128, 1024], F32, tag="oT")  # [d2 2][n 512]
            for h in range(NHL):
                hb = h // 2            # chunk pair index / d2 block
                po = (h % 2) * 64      # partition offset within chunk
                out_ps = acc_ps.tile([128, 512], F32, tag="acc",
                                     name="at_ps")  # rows 0:65
                pv = []  # deferred PV matmuls (one-strip software pipeline)
                for sg in range(8):    # strips of 2 j-chunks
                    ps = st_ps.tile([128, 1024], F32, tag="st")
                    for q2 in range(2):
                        jc = sg * 2 + q2
                        nc.tensor.matmul(
                            ps[:, q2 * 512:(q2 + 1) * 512],
                            _r(qkT[po:po + 64,
                                   (2 + hb) * N + jc * 128: (2 + hb) * N + (jc + 1) * 128]),
                            _r(qkT[po:po + 64,
                                   hb * N + it * 512: hb * N + (it + 1) * 512]),
                            start=True, stop=True)
                    strip = strip_p.tile([128, 1024], F32, tag="strip")
                    nc.scalar.activation(_r(strip[:]), ps[:], EXP, scale=SCALE)
                    if pv:
                        pv.pop(0)()
                    if sg == 2 and normq:
                        normq.pop(0)()
                    fill_hook(it, h, sg)

                    def _pv(strip=strip, sg=sg, h=h, out_ps=out_ps):
                        for q2 in range(2):
                            jc = sg * 2 + q2
                            nc.tensor.matmul(
                                out_ps[0:VW, :],
                                _r(vv[:, jc * NHL * VW + h * VW:
                                       jc * NHL * VW + h * VW + VW]),
                                _r(strip[:, q2 * 512:(q2 + 1) * 512]),
                                start=(jc == 0), stop=(jc == NT - 1))
                    pv.append(_pv)
                while pv:
                    pv.pop(0)()

                # normalize oT[d, i] = out[d, i] / out[64, i]; deferred one
                # head so the rank-1 broadcast matmul never stalls the PE
                def _norm(out_ps=out_ps, oT_t=oT_t, po=po, hb=hb):
                    rec = rec_p.tile([128, 512], F32, tag="rec")
                    with nc.allow_low_precision(reason="f32r view of f32"):
                        nc.vector.reciprocal(_r(rec[64:65, :]),
                                             out_ps[64:65, :])
                    bc = mm_ps.tile([128, 512], F32, tag="mm", name="bc_ps")
                    nc.tensor.matmul(bc[0:64, :], _r(ones[64:65, 0:64]),
                                     _r(rec[64:65, :]), start=True, stop=True)
                    nst = rec_p.tile([128, 512], F32, tag="nstg")
                    nc.vector.tensor_copy(nst[0:64, :], out_ps[0:64, :])
                    nc.vector.tensor_mul(
                        _r(oT_t[po:po + 64, hb * 512:(hb + 1) * 512]),
                        nst[0:64, :], bc[0:64, :])
                normq.append(_norm)
            for ng in range(4):
                for eh in range(2):
                    pending.append(
                        lambda oT_prev=oT_t, it_prev=it, ng=ng, eh=eh:
                        oproj_one(oT_prev, it_prev, ng, eh))
        while normq:
            normq.pop(0)()
        while pending:
            pending.pop(0)()
    if split_waits:
        _split_dma_waits(nc)
    return nc


def _split_dma_waits(nc):
    """walrus's DMA/LDWEIGHTS encodings take a single sync wait; move
    extra waits onto an EventSemaphore on the issuing sequencer."""
    fn = nc.m.functions[0]
    for bb in fn.blocks:
        insts = bb.instructions
        i = 0
        while i < len(insts):
            inst = insts[i]
            si = getattr(inst, "sync_info", None)
            if (si is not None and len(si.on_wait) > 1
                    and type(inst).__name__ != "InstEventSemaphore"):
                waits = list(si.on_wait)
                for k, w in enumerate(waits[:-1]):
                    ev = mybir.InstEventSemaphore(
                        name=f"{inst.name}-wsplit{k}", ins=[], outs=[])
                    ev.engine = inst.engine
                    ev.sync_info = type(si)(on_wait=[w], on_update=[])
                    insts.insert(i, ev)
                    i += 1
                inst.sync_info = type(si)(on_wait=waits[-1:],
                                          on_update=list(si.on_update))
            i += 1


_NC = None


def _get_nc():
    global _NC
    if _NC is None:
        _NC = build_nc()
    return _NC


def make_in_maps(x, w_qkv, w_out):
    x = np.asarray(x, dtype=np.float32)
    w_qkv = np.asarray(w_qkv, dtype=np.float32)
    w_out = np.asarray(w_out, dtype=np.float32)
    xT_by_b = [np.ascontiguousarray(x[b].T) for b in range(B)]
    in_maps = []
    for c in range(NCORES):
        b, g = divmod(c, 4)
        r0 = g * NHL * DH  # 256-wide feature slice
        wq = w_qkv[r0:r0 + EV]
        wk = w_qkv[DIM + r0:DIM + r0 + EV]
        wv = w_qkv[2 * DIM + r0:2 * DIM + r0 + EV]
        wT = np.ascontiguousarray(np.concatenate([wq, wk, wv], 0).T)
        woT = np.ascontiguousarray(w_out[:, r0:r0 + EV].T)
        in_maps.append({"xT": xT_by_b[b], "wT": wT, "woT": woT})
    return in_maps


def combine(results, b_out):
    """results: list of 8 dicts with 'out' [N, DIM] partials."""
    b_out = np.asarray(b_out, dtype=np.float32)
    out = np.empty((B, N, DIM), dtype=np.float32)
    for b in range(B):
        acc = results[4 * b]["out"].astype(np.float32, copy=True)
        for g in range(1, 4):
            acc += results[4 * b + g]["out"]
        out[b] = acc + b_out[None, :]
    return out


def kernel(x, w_qkv, w_out, b_out):
    from concourse.bass_utils import run_bass_kernel_spmd
    nc = _get_nc()
    in_maps = make_in_maps(x, w_qkv, w_out)
    res = run_bass_kernel_spmd(nc, in_maps, list(range(NCORES)))
    return combine(res.results, b_out)



# revision 42
# speedup vs baseline: 1.3357x; 1.3357x over previous
"""Trainium2 Bass kernel for MultiHeadAttention (B=2, N=2048, DIM=1024, H=16).

Sharding: 8 cores = 2 batches x 4 head-groups (4 heads each).
Each core computes qkv projections for its head slice, attention, and a
partial output projection (over its 256 d-columns). Host sums the 4
partials per batch and adds the bias.

All operands bf16 (PE runs 1 cycle/row at any moving size), PSUM fp32.

Device layout:
  xT  [d=1024, n=2048]  feature-major activations (pre-transposed, bf16)
  wT  [d=1024, e=768]   qkv weight slice (e = q|k|v 256 each, bf16)
  woT [d=256,  e=1024]  out-proj weight slice (bf16)
  qkT [ec 4][n]         q,k feature-major on chip (ec: q01,q23,k01,k23)
  vv  [j 128][nt][h][66] token-major V + ones column per head (softmax
      denominator comes out of the PV matmul as one extra moving column)
  S^T strips [j, i] in PSUM; exp on ACT in 2048-wide bf16 strips.
  PV reoriented: out[i 128, v 65] = strip_slice^T @ vv — full 128-partition
  output packing halves PV rows vs the [66, i] orientation. Normalization
  is a DVE reciprocal + broadcast multiply (per-partition denominators);
  a PE transpose (vs host identity) restores d-major for the out-proj.
"""

import os
import sys
from contextlib import ExitStack

import numpy as np

for _p in ("/opt/trn_rl_repo", os.path.expanduser("~/.axon_site/_ro/trn_rl_repo")):
    if os.path.isdir(_p) and _p not in sys.path:
        sys.path.append(_p)

import concourse.bass as bass  # noqa: E402
import concourse.mybir as mybir  # noqa: E402
import concourse.tile as tile  # noqa: E402

F32 = mybir.dt.float32
BF16 = mybir.dt.bfloat16
EXP = mybir.ActivationFunctionType.Exp

B, N, DIM, HEADS = 2, 2048, 1024, 16
DH = DIM // HEADS          # 64
NHL = 4                    # heads per core
SCALE = DH ** -0.5
NCORES = 8
E3 = 3 * NHL * DH          # 768 qkv features per core
EV = NHL * DH              # 256 v features per core
VW = DH + 2                # 66: v + ones columns
NT = N // 128              # 16 token chunks
DC = DIM // 128            # 8 d chunks


def build_nc(repeat=1, split_waits=True):
    nc = bass.Bass("TRN2", target_bir_lowering=False, debug=False,
                   num_devices=NCORES)
    xT_d = nc.dram_tensor("xT", [DIM, N], BF16, kind="ExternalInput").ap()
    wT_d = nc.dram_tensor("wT", [DIM, E3], BF16, kind="ExternalInput").ap()
    woT_d = nc.dram_tensor("woT", [EV, DIM], BF16, kind="ExternalInput").ap()
    id_d = nc.dram_tensor("ident", [128, 128], BF16, kind="ExternalInput").ap()
    out_d = nc.dram_tensor("out", [N, DIM], BF16, kind="ExternalOutput").ap()

    with tile.TileContext(nc) as tc, ExitStack() as ctx:
        if repeat > 1:
            ctx.enter_context(tc.For_i(0, repeat, 1))
        pers = ctx.enter_context(tc.tile_pool(name="pers", bufs=1))
        xT = pers.tile([128, DC, N], BF16, tag="xT", name="xT_sb")
        wT = pers.tile([128, DC, E3], BF16, tag="wT", name="wT_sb")
        woT = pers.tile([128, 2, DIM], BF16, tag="woT", name="woT_sb")
        qkT = pers.tile([128, 4, N], BF16, tag="qkT", name="qkT_sb")
        vv = pers.tile([128, NT, NHL, VW], BF16, tag="vv", name="vv_sb")
        ident = pers.tile([128, 128], BF16, tag="id", name="id_sb")

        strip_p = ctx.enter_context(tc.tile_pool(name="strip", bufs=16))
        oT_p = ctx.enter_context(tc.tile_pool(name="oT", bufs=2))
        ostg_p = ctx.enter_context(tc.tile_pool(name="ostg", bufs=4))
        rec_p = ctx.enter_context(tc.tile_pool(name="rec", bufs=2))
        norm_p = ctx.enter_context(tc.tile_pool(name="norm", bufs=2))

        # PSUM: st 2x4KB + mm 2x2KB + pv 2x2KB = 16KB (8 banks)
        st_ps = ctx.enter_context(tc.tile_pool(name="st_ps", bufs=2, space="PSUM"))
        mm_ps = ctx.enter_context(tc.tile_pool(name="mm_ps", bufs=2, space="PSUM"))
        pv_ps = ctx.enter_context(tc.tile_pool(name="pv_ps", bufs=2, space="PSUM"))

        # --- loads: direct bf16 DMA into persistent tiles (no round
        # copies needed — only fp32r operands require rounding producers).
        # Batched, just-in-time ordered: the k weights and token-quarter 0
        # of x land first so the k01/q01 projections for the first strips
        # finish ~11us in, letting the ACT exp pipeline (the binding
        # resource) start early; later x quarters land just before the
        # strips that consume them.
        nc.vector.memset(vv[:, :, :, DH:VW], 1.0)
        xr = xT_d.rearrange("(c p) n -> p c n", p=128)
        wr = wT_d.rearrange("(c p) e -> p c e", p=128)
        nc.sync.dma_start(out=wT[:, :, 256:512], in_=wr[:, :, 256:512])
        nc.sync.dma_start(out=xT[:, 0:4, 0:512], in_=xr[:, 0:4, 0:512])
        nc.sync.dma_start(out=wT[:, :, 0:256], in_=wr[:, :, 0:256])
        nc.sync.dma_start(out=xT[:, 4:8, 0:512], in_=xr[:, 4:8, 0:512])
        nc.sync.dma_start(out=ident[:], in_=id_d)
        nc.sync.dma_start(out=xT[:, :, 512:1024], in_=xr[:, :, 512:1024])
        nc.sync.dma_start(out=wT[:, :, 512:768], in_=wr[:, :, 512:768])
        nc.sync.dma_start(out=xT[:, :, 1024:1536], in_=xr[:, :, 1024:1536])
        nc.sync.dma_start(out=xT[:, :, 1536:2048], in_=xr[:, :, 1536:2048])
        nc.sync.dma_start(out=woT[:, :, :],
                          in_=woT_d.rearrange("(c p) e -> p c e", p=128))

        # --- q,k projections: qkT[ec][n], ec0,1=q heads01,23; ec2,3=k ---
        def qk_mm(ps_ap, ec, nt4, dc):
            nc.tensor.matmul(
                ps_ap,
                wT[:, dc, ec * 128:(ec + 1) * 128],
                xT[:, dc, nt4 * 512:(nt4 + 1) * 512],
                start=(dc == 0), stop=(dc == DC - 1))

        def v_proj(nt):
            ps = mm_ps.tile([128, NHL, DH], F32, tag="mm", name="v_ps")
            for dc in range(DC):
                nc.tensor.matmul(
                    ps[:, :, :],
                    xT[:, dc, nt * 128:(nt + 1) * 128],
                    wT[:, dc, 512:768],
                    start=(dc == 0), stop=(dc == DC - 1))
            nc.vector.tensor_copy(vv[:, nt, :, 0:DH], ps[:, :, :])

        # k01/q01 for the first strips run before the attention loop; the
        # remaining projection groups (k01 quarters 1-3, k23, v, late
        # q01/q23 slices) are interleaved into the strip loop via the
        # fill schedule below, so the exp pipeline (the binding ACT
        # resource) starts ~11us in and the PE fills its slack with
        # projection work.
        def _k01q(q):
            ps = mm_ps.tile([128, 512], F32, tag="mm", name="k01_ps")
            sl = ps[:]
            for dc in range(DC):
                qk_mm(sl, 2, q, dc)
            nc.vector.tensor_copy(qkT[:, 2, q * 512:q * 512 + 256], sl[:, 0:256])
            nc.vector.tensor_copy(qkT[:, 2, q * 512 + 256:(q + 1) * 512],
                                  sl[:, 256:512])

        def _g(ec, nt4):
            ps = mm_ps.tile([128, 512], F32, tag="mm", name="qk_ps")
            for dc in range(DC):
                qk_mm(ps[:], ec, nt4, dc)
            nc.vector.tensor_copy(qkT[:, ec, nt4 * 512:(nt4 + 1) * 512], ps[:])

        # prologue: k01 quarter 0 and the it0 q01 slice, dc-interleaved so
        # both finish right as the last quarter-0 x chunk lands
        k01_ps = st_ps.tile([128, 2, 512], F32, tag="st", name="k01_ps")
        q01_ps = mm_ps.tile([128, 512], F32, tag="mm", name="q01_ps")
        for dc in range(DC):
            qk_mm(k01_ps[:, 0, :], 2, 0, dc)
            qk_mm(q01_ps[:], 0, 0, dc)
        nc.vector.tensor_copy(qkT[:, 2, 0:256], k01_ps[:, 0, 0:256])
        nc.vector.tensor_copy(qkT[:, 2, 256:512], k01_ps[:, 0, 256:512])
        nc.vector.tensor_copy(qkT[:, 0, 0:512], q01_ps[:])

        # --- attention + out-projection, per i-tile of 512 queries ---
        def oproj_one(oT_prev, it_prev, ng, eh, use_act=False, pool=None):
            ps = (pool or mm_ps).tile([128, 512], F32,
                                      tag="pv" if pool is pv_ps else "mm",
                                      name="op_ps")
            for d2 in range(2):
                nc.tensor.matmul(
                    ps[:],
                    oT_prev[:, d2, ng * 128:(ng + 1) * 128],
                    woT[:, d2, eh * 512:(eh + 1) * 512],
                    start=(d2 == 0), stop=(d2 == 1))
            stg = ostg_p.tile([128, 512], BF16, tag="ostg")
            if use_act:
                nc.scalar.activation(stg[:], ps[:],
                                     mybir.ActivationFunctionType.Copy)
            else:
                nc.vector.tensor_copy(stg[:], ps[:])
            nc.sync.dma_start(
                out=out_d[it_prev * 512 + ng * 128: it_prev * 512 + (ng + 1) * 128,
                          eh * 512:(eh + 1) * 512],
                in_=stg[:])

        # Global software pipeline over 128 strip units (16 windows x 8
        # strips). Strips emit back to back (the 2-slot st psum rotation
        # paces the PE against ACT automatically); projection and out-proj
        # fills pop between strips only while the PE's emitted work trails
        # the ACT timeline (cost-model balancer), with hard deadlines where
        # a fill produces data a later strip/PV reads (emission order IS
        # the dependency order for same-region reads).
        # Fills pop after a fixed unit's ACT emission, placed just ahead
        # of their first consumer: v_proj(nt) before the PV that reads
        # vv[nt]; k01 quarter q before the S of sg=2q; k23/q23 in the h1
        # window ahead of their h2 consumers; the q01/q23 slices for it+1
        # late in it's ACT-bound h2/h3 windows.
        fillq = []
        fillq.append((0, lambda: _k01q(1)))
        fillq.append((2, lambda: _k01q(2)))
        fillq.append((4, lambda: _k01q(3)))
        for nt in range(NT):
            fillq.append((nt // 2, lambda nt=nt: v_proj(nt)))
        fillq.append((8, lambda: _g(1, 0)))      # q23 it0
        for q in range(4):
            fillq.append((9 + q, lambda q=q: _g(3, q)))
        for it in range(1, 4):
            fillq.append((32 * it - 11, lambda it=it: _g(0, it)))
            fillq.append((32 * it + 5, lambda it=it: _g(1, it)))
        fillq.sort(key=lambda f: f[0])

        pending = []  # out-proj units, popped in later its' strip windows
        normq = []    # deferred per-head normalize+transpose chains

        def pop_fills(u):
            while fillq and fillq[0][0] <= u:
                fillq.pop(0)[1]()
            if u % 2 == 1 and pending and pending[0][0] <= u:
                pending.pop(0)[1]()

        def make_norm(pv_t, oT_t, po, hb):
            def _norm():
                rec = rec_p.tile([128, NHL, 1], F32, tag="rec")
                nc.vector.reciprocal(rec[:], pv_t[0][:, :, DH:DH + 1])
                nrm = norm_p.tile([128, NHL, DH], BF16, tag="nrm")
                in1, in2 = bass.broadcast_tensor_aps(pv_t[0][:, :, 0:DH],
                                                     rec[:])
                nc.vector.tensor_mul(nrm[:], in1, in2)
                tp = mm_ps.tile([128, 512], BF16, tag="mm", name="tp_ps")
                for isub in range(4):
                    nc.tensor.transpose(
                        tp[0:64, isub * 128:(isub + 1) * 128],
                        nrm[:, isub, :], ident[:])
                nc.vector.tensor_copy(oT_t[po:po + 64, hb, :], tp[0:64, :])
            return _norm

        # PV runs one window behind, one isub at a time: PSUM accumulation
        # groups are exclusive per 2KB bank, so the 4 output i-subtiles
        # accumulate sequentially (start..stop per isub over all 16
        # j-chunks), reading the previous window's strips, which the
        # 16-deep strip pool keeps alive. The last isub chains the
        # normalize+transpose for its window.
        def make_pv(strips, h, pv_t, isub, last=None):
            def _pvi():
                if isub == 0:
                    pv_t[0] = pv_ps.tile([128, NHL, VW], F32, tag="pv",
                                         name="pv_ps_t")  # [i, isub, v|den]
                for jc in range(NT):
                    nc.tensor.matmul(
                        pv_t[0][:, isub, 0:DH + 1],
                        strips[jc // 2][:, jc % 2, isub * 128:(isub + 1) * 128],
                        vv[:, jc, h, 0:DH + 1],
                        start=(jc == 0), stop=(jc == NT - 1))
                if last is not None:
                    last()
            return _pvi

        pvq = []     # per-(window, isub) PV units + chained norm
        strips_w = []
        oT_t = None
        for u in range(128):
            it, r = divmod(u, 32)
            h, sg = divmod(r, 8)
            hb = h // 2            # head-pair index (q01/q23, k01/k23)
            po = (h % 2) * 64      # partition offset within pair
            if r == 0:
                oT_t = oT_p.tile([128, 2, 512], BF16, tag="oT", name="oT_t")
            ps = st_ps.tile([128, 2, 512], F32, tag="st", name="s_ps")
            for q2 in range(2):
                jc = sg * 2 + q2
                nc.tensor.matmul(
                    ps[:, q2, :],
                    qkT[po:po + 64, 2 + hb, jc * 128:(jc + 1) * 128],
                    qkT[po:po + 64, hb, it * 512:(it + 1) * 512],
                    start=True, stop=True)
            strip = strip_p.tile([128, 2, 512], BF16, tag="strip")
            nc.scalar.activation(strip[:], ps[:], EXP, scale=SCALE)
            strips_w.append(strip)
            if sg % 2 == 1 and pvq:
                pvq.pop(0)()
            pop_fills(u)

            if sg == 7:
                pv_t = [None]
                norm = make_norm(pv_t, oT_t, po, hb)
                for isub in range(4):
                    pvq.append(make_pv(list(strips_w), h, pv_t, isub,
                                       norm if isub == 3 else None))
                strips_w.clear()
                if h == 3:
                    # out-proj must not emit before norm(it, h3), which
                    # chains onto the last PV unit of window 4it+3,
                    # popping during window 4it+4
                    min_u = 32 * it + 40
                    for ng in range(4):
                        for eh in range(2):
                            pending.append((min_u,
                                lambda oT_prev=oT_t, it_prev=it, ng=ng,
                                eh=eh, use_act=False, pool=None:
                                oproj_one(oT_prev, it_prev, ng, eh,
                                          use_act, pool)))
        while pvq:
            pvq.pop(0)()
        # final flush: after the last strip ACT is idle and the pv psum
        # slots are free — alternate the staging copies between ACT and
        # DVE and the psum tiles between the mm and pv pools so the
        # remaining out-proj groups pipeline 2-wide
        for i, (_, p) in enumerate(pending):
            p(use_act=(i % 2 == 1), pool=(pv_ps if i % 2 == 1 else mm_ps))
        pending.clear()
    if split_waits:
        _split_dma_waits(nc)
    return nc


def _split_dma_waits(nc):
    """walrus's DMA/LDWEIGHTS encodings take a single sync wait; move
    extra waits onto an EventSemaphore on the issuing sequencer."""
    fn = nc.m.functions[0]
    for bb in fn.blocks:
        insts = bb.instructions
        i = 0
        while i < len(insts):
            inst = insts[i]
            si = getattr(inst, "sync_info", None)
            if (si is not None and len(si.on_wait) > 1
                    and type(inst).__name__ != "InstEventSemaphore"):
                waits = list(si.on_wait)
                for k, w in enumerate(waits[:-1]):
                    ev = mybir.InstEventSemaphore(
                        name=f"{inst.name}-wsplit{k}", ins=[], outs=[])
                    ev.engine = inst.engine
                    ev.sync_info = type(si)(on_wait=[w], on_update=[])
                    insts.insert(i, ev)
                    i += 1
                inst.sync_info = type(si)(on_wait=waits[-1:],
                                          on_update=list(si.on_update))
            i += 1


_NC = None


def _get_nc():
    global _NC
    if _NC is None:
        _NC = build_nc()
    return _NC


def make_in_maps(x, w_qkv, w_out):
    import ml_dtypes
    bf16 = ml_dtypes.bfloat16
    x = np.asarray(x, dtype=np.float32)
    w_qkv = np.asarray(w_qkv, dtype=np.float32)
    w_out = np.asarray(w_out, dtype=np.float32)
    xT_by_b = [np.ascontiguousarray(x[b].T).astype(bf16) for b in range(B)]
    ident = np.eye(128, dtype=bf16)
    in_maps = []
    for c in range(NCORES):
        b, g = divmod(c, 4)
        r0 = g * NHL * DH  # 256-wide feature slice
        wq = w_qkv[r0:r0 + EV]
        wk = w_qkv[DIM + r0:DIM + r0 + EV]
        wv = w_qkv[2 * DIM + r0:2 * DIM + r0 + EV]
        wT = np.ascontiguousarray(np.concatenate([wq, wk, wv], 0).T).astype(bf16)
        woT = np.ascontiguousarray(w_out[:, r0:r0 + EV].T).astype(bf16)
        in_maps.append({"xT": xT_by_b[b], "wT": wT, "woT": woT, "ident": ident})
    return in_maps


def combine(results, b_out):
    """results: list of 8 dicts with 'out' [N, DIM] partials."""
    b_out = np.asarray(b_out, dtype=np.float32)
    out = np.empty((B, N, DIM), dtype=np.float32)
    for b in range(B):
        acc = results[4 * b]["out"].astype(np.float32)
        for g in range(1, 4):
            acc += results[4 * b + g]["out"].astype(np.float32)
        out[b] = acc + b_out[None, :]
    return out


def kernel(x, w_qkv, w_out, b_out):
    from concourse.bass_utils import run_bass_kernel_spmd
    nc = _get_nc()
    in_maps = make_in_maps(x, w_qkv, w_out)
    res = run_bass_kernel_spmd(nc, in_maps, list(range(NCORES)))
    return combine(res.results, b_out)
